# revision 63
# baseline (speedup 1.0000x reference)
"""Trainium2 Bass kernel for batched Hadamard transform.

out = (x_re + i*x_im) @ H4096 with H4096 real (entries +-1/64), so
out_re = x_re @ H and out_im = x_im @ H independently.

Algorithm: H4096 = H16 (x) H256 (Kronecker, n = 256u + v).  Viewing each
4096-row as V[u, v] (16x256), the transform is W = (H16/4) V (H256/16).
Key trick: a normal PE matmul computes lhsT.T @ rhs with lhsT stationary,
so using the DATA as the stationary operand fuses the transpose into the
matmul.  A tile packs 8 rows: partition (a, u) = 16a + u, cols v.
    MM1 (per v-half q): out1_q = A[:, vhalf_q].T @ S1 (S1 = blkdiag H16/4 x8)
    MM2 (accumulate over q): out2 += out1_q.T @ S2_q  (S2_q = H256 half /16)
out2 lands back in row-major layout, no separate transposes.  The v=256
split makes every DMA descriptor line 256 elements = 512 bytes in bf16
(full SDMA line rate; 128-element lines pay a 2x small-descriptor
penalty).  All I/O and SBUF residency in bf16 (tolerance is 2e-2; bf16
path measures ~4e-3), PSUM accumulation in fp32.

Pipeline per 64-row DMA batch: 4 quarter-DMAs in (SP/HWDGE) ->
per 32-row quad {8x MM1 -> PSUM, DVE copy [128,1024] -> bf16 SBUF,
8x MM2 -> PSUM, 2x ACT copy -> bf16 SBUF} -> quarter out-DMAs issued
behind each copy2 (gpsimd/SWDGE; final one on the idle SP HWDGE ring).
The emission is software-pipelined one quad deep (engines execute
their queues in program order, so MM1s of quad g+1 are emitted before
MM2s of quad g to keep the PE fed during the copy1 latency).
Head/tail trims: ~1us PE warm-up burst under the first DMA (HAM ramp),
1-tile first transfer, split first/last copies, last copy2 on DVE.
Engine budgets per core (cost model): PE 42.7us busy / 98%-occupied
window, DVE 39us, ACT 41us, SP 33us, Pool 32us -> 47.7us/core vs 307us
for the fp32 H64xH64 matmul+transpose baseline (6.4x).

Sharding: data-parallel over batch (8 batches -> 8 NeuronCores).
"""

import re
import numpy as np
import ml_dtypes

from concourse import bass, tile
import concourse.mybir as mybir
from concourse.bass_utils import run_bass_kernel_spmd
from concourse.tile import TileContext
from concourse.tile_sem_assignment import tick_to_sem


def _drain_and_barrier_split(self, tick_clock, wait_clock):
    # The stock kernel-tail drain carries one sem-wait per active proc on a
    # single instruction; this walrus build rejects >2 sync waits per
    # instruction ("Too many sync wait commands").  Emit one wait_ge per
    # proc instead, then a bare drain.
    gc = tick_clock.global_clock
    ticks = [int(v) for v in re.findall(r"\d+", repr(gc))]
    for proc, sem in sorted(self.sems.allocated().items()):
        if proc < len(ticks) and ticks[proc] > 0:
            self.nc.sync.wait_ge(sem, tick_to_sem(ticks[proc], proc))
    self.nc.sync.drain()
    self.nc.all_engine_barrier()
    assert self.sems is not None
    popped = self.nc._tile_sem_poison_stack.pop()
    assert popped is self._sem_poison
    self.nc.clear_and_free_semaphores(list(self.sems.allocated().values()))
    self.nc.all_engine_barrier()


TileContext._drain_and_barrier = _drain_and_barrier_split

_MAX_WAITS = 1


def _split_excess_waits(nc):
    """This walrus build rejects instructions with >2 sync-wait commands.
    Move excess waits onto same-engine NoOps inserted just before the
    instruction (engines execute their queue in order, so the sync semantics
    are preserved)."""
    n_split = 0
    for fn in nc.m.functions:
        for bb in fn.blocks:
            insts = list(bb.instructions)
            out = []
            for inst in insts:
                si = inst.sync_info
                waits = list(si.on_wait) if si and si.on_wait else []
                if len(waits) > _MAX_WAITS:
                    extra = waits[: len(waits) - _MAX_WAITS]
                    keep = waits[len(waits) - _MAX_WAITS :]
                    for ci in range(0, len(extra), _MAX_WAITS):
                        chunk = extra[ci : ci + _MAX_WAITS]
                        n_split += 1
                        nop = mybir.InstNoOp(
                            name=f"waitnop-{n_split}-{inst.name}",
                            engine=inst.engine,
                            sync_info=mybir.SyncInfo(
                                on_wait=list(chunk), on_update=[]
                            ),
                        )
                        out.append(nop)
                    inst.sync_info = mybir.SyncInfo(
                        on_wait=list(keep), on_update=list(si.on_update)
                    )
                out.append(inst)
            if len(out) != len(insts):
                bb.instructions = out
    return n_split


B, M, N = 8, 512, 4096
NCORES = 8
F32 = mybir.dt.float32
BF16 = mybir.dt.bfloat16
BF = ml_dtypes.bfloat16

NB = 8      # DMA batches per tensor (64 rows each)
TPB = 8     # 8-row tiles per DMA batch
# A/C tiles are padded past 256*TPB cols so the DMA access pattern's
# partition-step dim (stride = tile width) can't be merged with the
# tile-step col dim (extent 256*TPB) into one flat dim -- that merge
# destroys the partition structure of the transfer.
WPAD = 256 * TPB + 256


def _hadamard(n: int) -> np.ndarray:
    h = np.array([[1.0]], dtype=np.float64)
    while h.shape[0] < n:
        h = np.block([[h, h], [h, -h]])
    return h


def _host_constants():
    # W = (H16/4) V (H256/16) -> combined scale 1/64 matches H4096 entries.
    h16 = (_hadamard(16) / 4.0).astype(BF)
    h256 = (_hadamard(256) / 16.0).astype(BF)
    s1 = np.zeros((128, 128), dtype=BF)
    for a in range(8):
        s1[16 * a : 16 * a + 16, 16 * a : 16 * a + 16] = h16
    # s2[w, 256q + l] = H256[128q + w, l] / 16   (the two halves side by side)
    s2 = np.zeros((128, 512), dtype=BF)
    s2[:, 0:256] = h256[0:128, :]
    s2[:, 256:512] = h256[128:256, :]
    return s1, s2


def _build():
    nc = bass.Bass()
    xre = nc.dram_tensor("x_re", [M, N], BF16, kind="ExternalInput")
    xim = nc.dram_tensor("x_im", [M, N], BF16, kind="ExternalInput")
    s1 = nc.dram_tensor("s1", [128, 128], BF16, kind="ExternalInput")
    s2 = nc.dram_tensor("s2", [128, 512], BF16, kind="ExternalInput")
    ore = nc.dram_tensor("o_re", [M, N], BF16, kind="ExternalOutput")
    oim = nc.dram_tensor("o_im", [M, N], BF16, kind="ExternalOutput")

    with tile.TileContext(nc) as tc:
        with (
            tc.tile_pool(name="const", bufs=1) as cpool,
            tc.tile_pool(name="a", bufs=4) as apool,
            tc.tile_pool(name="b", bufs=4) as bpool,
            tc.tile_pool(name="c", bufs=3) as opool,
            tc.tile_pool(name="ps1", bufs=2, space="PSUM") as ps1pool,
            tc.tile_pool(name="ps2", bufs=3, space="PSUM") as ps2pool,
        ):
            s1_sb = cpool.tile([128, 128], BF16)
            s2_sb = cpool.tile([128, 512], BF16)
            nc.scalar.dma_start(s1_sb[:], s1[:])
            nc.scalar.dma_start(s2_sb[:], s2[:])

            # PE warm-up while the first input DMA is in flight (HAM ramp)
            warm = cpool.tile([128, 128], BF16)
            nc.vector.memset(warm[:], 0.0)
            wps = ps2pool.tile([128, 512], F32, tag="ps2", name="ps2")
            for _ in range(9):
                nc.tensor.matmul(
                    wps[:, :128], warm[:], warm[:], start=True, stop=True
                )


            ctxs = []
            for xt, ot in ((xre, ore), (xim, oim)):
                # x[r0 + 8t + a, 256u + v] -> A[16a + u, 256t + v]
                # (h splits each DMA batch into quarters of TPB/4 tiles so
                # transfers pipeline at finer grain against compute)
                xv = xt[:].rearrange(
                    "(b h t a) (u v) -> b h (a u) t v",
                    b=NB, h=4, t=TPB // 4, a=8, u=16, v=256,
                )
                # o[r0 + 8t + a, 256m + l] <- C[16a + m, 256t + l]
                ov = ot[:].rearrange(
                    "(b h t a) (m l) -> b h (a m) t l",
                    b=NB, h=4, t=TPB // 4, a=8, m=16, l=256,
                )
                ctxs.append((xt is xre, xv, ov))

            # Flatten to a global quad list and software-pipeline the PE
            # queue: engines execute their queue in program order, so MM2s
            # of quad g would otherwise block MM1s of quad g+1 while
            # waiting on copy1(g).  Emitting MM1s one quad ahead keeps the
            # PE fed during the copy latency.
            QW = 256 * (TPB // 4)
            NQ = TPB // 4  # quads per batch
            quads = []
            for first_t, xv, ov in ctxs:
                for bb in range(NB):
                    for j in range(NQ):
                        quads.append((first_t, xv, ov, bb, j))
            n_g = len(quads)
            batch_tiles = {}
            pending = []

            def emit_front(g):
                first_t, xv, ov, bb, j = quads[g]
                if j == 0:
                    A = apool.tile([128, WPAD], BF16, tag="A", name="A")
                    C = opool.tile([128, WPAD], BF16, tag="C", name="C")
                    batch_tiles[(first_t, bb)] = (A, C)
                    for h in range(4):
                        dst = A[:, h * QW : (h + 1) * QW].rearrange(
                            "p (t v) -> p t v", t=TPB // 4, v=256
                        )
                        if g == 0 and h == 0:
                            # 1-tile first transfer: compute starts sooner
                            for tt in range(TPB // 4):
                                nc.sync.dma_start(
                                    dst[:, tt : tt + 1], xv[bb, h][:, tt : tt + 1]
                                )
                        else:
                            nc.sync.dma_start(dst, xv[bb, h])
                A, C = batch_tiles[(first_t, bb)]
                ps1 = ps1pool.tile([128, 1024], F32, tag="ps1", name="ps1")
                for k in range(4):
                    tau = 4 * j + k
                    for q in range(2):
                        nc.tensor.matmul(
                            ps1[:, 256 * k + 128 * q : 256 * k + 128 * q + 128],
                            A[:, 256 * tau + 128 * q : 256 * tau + 128 * q + 128],
                            s1_sb[:],
                            start=True,
                            stop=True,
                        )
                return (g, ps1)

            def emit_back(g, ps1):
                first_t, xv, ov, bb, j = quads[g]
                A, C = batch_tiles[(first_t, bb)]
                Bt = bpool.tile([128, 1024], BF16, tag="Bt", name="Bt")
                if g == 0 or g == n_g - 1:
                    # halve the pipeline fill/drain bubbles
                    nc.vector.tensor_copy(Bt[:, :512], ps1[:, :512])
                    nc.vector.tensor_copy(Bt[:, 512:], ps1[:, 512:])
                else:
                    nc.vector.tensor_copy(Bt[:], ps1[:])
                for k2 in range(2):
                    ps2 = ps2pool.tile([128, 512], F32, tag="ps2", name="ps2")
                    for k in range(2):
                        for q in range(2):
                            nc.tensor.matmul(
                                ps2[:, 256 * k : 256 * k + 256],
                                Bt[:, 512 * k2 + 256 * k + 128 * q : 512 * k2 + 256 * k + 128 * q + 128],
                                s2_sb[:, 256 * q : 256 * q + 256],
                                start=(q == 0),
                                stop=(q == 1),
                            )
                    if g == n_g - 1 and k2 == 1:
                        nc.vector.tensor_copy(
                            C[:, 1024 * j + 512 * k2 : 1024 * j + 512 * k2 + 512],
                            ps2[:],
                        )
                    else:
                        nc.scalar.copy(
                            C[:, 1024 * j + 512 * k2 : 1024 * j + 512 * k2 + 512],
                            ps2[:],
                        )
                    h = 2 * j + k2
                    eng = (
                        nc.sync
                        if (g == n_g - 1 and h == 3)
                        else nc.gpsimd
                    )
                    eng.dma_start(
                        ov[bb, h],
                        C[:, h * QW : (h + 1) * QW].rearrange(
                            "p (t l) -> p t l", t=TPB // 4, l=256
                        ),
                    )

            for g in range(n_g):
                pending.append(emit_front(g))
                if len(pending) > 1:
                    emit_back(*pending.pop(0))
            while pending:
                emit_back(*pending.pop(0))

    _split_excess_waits(nc)
    return nc


_NC_CACHE = {}


def _get_nc():
    if "nc" not in _NC_CACHE:
        _NC_CACHE["nc"] = _build()
    return _NC_CACHE["nc"]


def _run(x_re: np.ndarray, x_im: np.ndarray, trace: bool = False):
    nc = _get_nc()
    s1, s2 = _host_constants()
    xre_b = np.asarray(x_re, dtype=np.float32).astype(BF)
    xim_b = np.asarray(x_im, dtype=np.float32).astype(BF)
    in_maps = []
    for b in range(NCORES):
        in_maps.append(
            {
                "x_re": np.ascontiguousarray(xre_b[b]),
                "x_im": np.ascontiguousarray(xim_b[b]),
                "s1": s1,
                "s2": s2,
            }
        )
    res = run_bass_kernel_spmd(nc, in_maps, list(range(NCORES)), trace=trace)
    return res


def kernel(x_re, x_im):
    res = _run(x_re, x_im, trace=False)
    out = np.empty((B, M, N), dtype=np.complex64)
    for b in range(NCORES):
        out.real[b] = np.asarray(res.results[b]["o_re"], dtype=np.float32)
        out.imag[b] = np.asarray(res.results[b]["o_im"], dtype=np.float32)
    return out
